# revision 5
# baseline (speedup 1.0000x reference)
"""Trainium2 Bass kernel for ChannelAttention1D.

Inputs (full): x (8, 256, 16384) f32, gamma (1,) f32.
  energy = einsum('bit,bjt->bij', x, x)
  att    = softmax(max_j(energy) - energy, axis=-1)
  out    = gamma * einsum('bij,bjt->bit', att, x) + x

Sharding: data-parallel over B across 8 NeuronCores (one batch per core).

HBM traffic is the roofline (memory regime): x is shipped once as fp16
(8 MiB/core) and the output is written as fp16 (8 MiB/core, upcast to f32
on the host).  The fp16 I/O rounding (~5e-4 max rel err) is far inside the
2e-2 gate; with gamma == 0 (the shipped input distribution) the folded
attention operand is exactly the identity, so out == fp16(x) bit-exact.

DMA layouts are chunked so descriptors stay large (descriptor generation
on the DGE caps DMA below the 358 GB/s wire rate when rows are only
4 KiB): input and output segments are separate DRAM tensors with 2-16 KiB
rows (small first input segment so compute starts early, small last
output segments to shorten the drain tail).  The host packs/unpacks.

Softmax epilogue (vs the earlier revision): G01^T is PE-transposed
directly into the same PSUM tile that accumulates G11, so row-block 1's
energy row is one contiguous [G01^T | G11] tile and its chain collapses
to a single rowmin + single exp (the row-0 shape) -- about 5 us of
serial cross-engine hops removed from the softmax window.

Energy matmuls are emitted one transpose h-group behind (not one
segment behind): the PE FIFO then always holds ready energy work instead
of data-starved next-segment transposes, which removes ~8 us of PE
head-of-line idle during the input stream and shrinks the post-stream
backlog to one h-group.  The last input segment is small (2048) so the
energy tail after the final byte is short.

Phase 2 is interleaved into the softmax epilogue: with 512-wide
phase-2 psum tiles (1 bank each), pe(2) + pt(2) + po(4) PSUM banks fit
simultaneously, so the first m=0 output block's matmuls/drains/writes
are emitted right after aT[0] -- phase 2 and the output write stream
start while the row-1 softmax chain still runs, and the early matmuls
keep the PE HAM-warm (the dummy clock-hold transposes are gone).

Per-core pipeline (C=256, T=16384):
  phase 1: sync-ring DMA streams x fp16 segments.  PE transposes 128x128
           blocks into PSUM (fp16); DVE (m=0) and Act (m=1) copy them to
           SBUF downcasting to fp8e4m3 in DoubleRow-pair layout
           xtp [128 tp, q, 2 kt, 2 m, 128 c].  Energy accumulates with
           fp8 DoubleRow matmuls (K=256 per pass): only G00|G01 (pe0) and
           G11 (pe1) are computed; G10 = G01^T by symmetry.
  softmax: att = exp(rowmin - energy) / rowsum (== softmax(rowmax -
           energy)); G01^T is reconstructed with an fp16 PE transpose.
           A = gamma*att/rowsum + I is formed directly (identity folded
           into the operand), so phase 2 needs no residual add.
  phase 2: out = A.T-transposed matmuls @ x straight from the resident
           natural x tiles (fp16), PSUM drained to fp16 by DVE/Act
           alternately, 16 KiB-row writeback.
"""

import os

# Neuron cores can be left in a degraded (down-clocked ~20%) state by prior
# runs; a core reset at runtime init restores full clocks (see trn2 pitfalls).
os.environ.setdefault("NEURON_RT_RESET_CORES", "1")

import numpy as np

import concourse.bacc as bacc
import concourse.bass as bass
import concourse.mybir as mybir
import concourse.tile as tile
from concourse.bass_utils import run_bass_kernel_spmd

F32 = mybir.dt.float32
F16 = mybir.dt.float16
F8 = mybir.dt.float8e4

B = 8
C = 256
T = 16384
N_CORES = 8
SEGS = [2048, 4096, 4096, 4096, 2048]   # in segments (fp16 cols) per m
QMAX = max(SEGS) // 256                 # xtp tile q capacity (padded)
W2 = 1024            # phase-2 psum tile width (2 fp32 PSUM banks)
WO = 8192            # phase-2 output staging width (16 KiB rows)

LAST_RESULTS = None  # BassKernelResults of the most recent run (for test.py)


def _build_nc():
    nc = bacc.Bacc(
        "TRN2",
        target_bir_lowering=False,
        debug=False,
        enable_asserts=False,
        num_devices=N_CORES,
    )
    seg_d = [
        nc.dram_tensor(f"xseg{i}", [2, 128, w], F16, kind="ExternalInput")
        for i, w in enumerate(SEGS)
    ]
    id_d = nc.dram_tensor("identity", [128, 128], F16, kind="ExternalInput")
    g_d = nc.dram_tensor("gamma_b", [128, 1], F32, kind="ExternalInput")
    o_d = nc.dram_tensor("out", [2, T // WO, 128, WO], F16, kind="ExternalOutput")

    Exp = mybir.ActivationFunctionType.Exp
    Copy = mybir.ActivationFunctionType.Copy
    Alu = mybir.AluOpType
    X = mybir.AxisListType.X
    DR = mybir.MatmulPerfMode.DoubleRow
    NQ = T // 256

    with tile.TileContext(nc) as tc:
        with (
            tc.tile_pool(name="xh", bufs=1) as xhpool,
            tc.tile_pool(name="xtp", bufs=3) as xtppool,
            tc.tile_pool(name="sm", bufs=1) as smpool,
            tc.tile_pool(name="outp", bufs=4) as outpool,
        ):
            ident = smpool.tile([128, 128], F16, tag="ident", name="ident")
            nc.scalar.dma_start(ident[:], id_d.ap())
            g128 = smpool.tile([128, 1], F32, tag="g128", name="g128")
            nc.scalar.dma_start(g128[:], g_d.ap())

            # Resident fp16 x (natural layout), one tile per 128-row block.
            xh = [
                xhpool.tile([128, T], F16, tag=f"xh{m}", name=f"xh{m}")
                for m in range(2)
            ]

            with tc.tile_pool(
                name="pe", bufs=1, space=bass.MemorySpace.PSUM
            ) as pepool:
                ptx_ctx = tc.tile_pool(
                    name="ptx", bufs=4, space=bass.MemorySpace.PSUM
                )
                ptxpool = ptx_ctx.__enter__()
                pe0 = pepool.tile([128, C], F32, tag="pe0", name="pe0")
                pe1b = pepool.tile([128, C], F32, tag="pe1b", name="pe1b")
                pe1 = pe1b[:, 128:256]


                # ---- phase 1: stream in, PE-transpose, fp8 DR energy ----
                # energy matmuls run one segment behind the transposes so the
                # PE never stalls waiting for the current segment's DVE/Act
                # psum->sbuf copies
                k = 0
                off = 0
                pending = []  # [(xtp, q0, nq), ...] per ready h-group

                def emit_energy(xtp, q0, nq):
                    nonlocal k
                    for q in range(q0, q0 + nq):
                        st = k == 0
                        sp = k == NQ - 1
                        w0 = xtp[:, q, :, 0, :]
                        w1 = xtp[:, q, :, 1, :]
                        rhs_all = xtp[:, q].rearrange("p kt m c -> p kt (m c)")
                        nc.tensor.matmul(
                            pe0[:], w0, rhs_all, start=st, stop=sp, perf_mode=DR
                        )
                        nc.tensor.matmul(
                            pe1, w1, w1, start=st, stop=sp, perf_mode=DR
                        )
                        k += 1

                for si, w in enumerate(SEGS):
                    for m in range(2):
                        nc.sync.dma_start(
                            xh[m][:, off:off + w], seg_d[si].ap()[m]
                        )
                    # xtp[p, q, kt, m, c] = x[m*128+c, off + (2q+kt)*128 + p]
                    xtp = xtppool.tile(
                        [128, QMAX, 2, 2, 128], F8, tag="xtp", name=f"xtp{si}"
                    )
                    ntb = w // 128
                    for h in range((ntb + 7) // 8):
                        tbs = min(8, ntb - h * 8)
                        for m in range(2):
                            ptx = ptxpool.tile(
                                [128, 8, 128], F16, tag="ptx",
                                name=f"ptx{m}_{si}_{h}"
                            )
                            for tbl in range(tbs):
                                tb = h * 8 + tbl
                                nc.tensor.transpose(
                                    ptx[:, tbl, :],
                                    xh[m][:, off + tb * 128:off + (tb + 1) * 128],
                                    ident[:],
                                )
                            src = ptx[:, 0:tbs, :].rearrange(
                                "p (q kt) c -> p q kt c", kt=2
                            )
                            dst = xtp[:, h * 4:h * 4 + tbs // 2, :, m, :]
                            if m == 0:
                                nc.vector.tensor_copy(dst, src)
                            else:
                                nc.scalar.activation(dst, src, Copy)
                        pending.append((xtp, h * 4, tbs // 2))
                        if len(pending) > 1:
                            emit_energy(*pending.pop(0))
                    off += w
                for p in pending:
                    emit_energy(*p)

                # G01^T goes straight into pe1b[:, 0:128] so row-1''s
                # energy row is one contiguous psum tile (one reduce+exp)
                s01 = smpool.tile([128, 128], F32, tag="s01", name="s01")
                nc.vector.tensor_copy(s01[:], pe0[:, 128:256])
                id32 = smpool.tile([128, 128], F32, tag="id32", name="id32")
                nc.scalar.activation(id32[:], ident[:], Copy)
                nc.tensor.transpose(pe1b[:, 0:128], s01[:], id32[:])

                ptx_ctx.__exit__(None, None, None)

                # ---- softmax epilogue; A = gamma*att/rowsum + I ----
                att16 = [
                    smpool.tile([128, C], F16, tag=f"a{m}", name=f"a{m}")
                    for m in range(2)
                ]
                aT = []  # fp16 A.T operands for phase 2, [128 j, 2 jb, 128 i]
                with tc.tile_pool(
                    name="pt", bufs=2, space=bass.MemorySpace.PSUM
                ) as ptpool:
                    # row block 0: energy row = pe0 = [G00 | G01]
                    e0 = smpool.tile([128, C], F32, tag="e0", name="e0")
                    rs0 = smpool.tile([128, 1], F32, tag="rs0", name="rs0")
                    rm0 = smpool.tile([128, 1], F32, tag="rm0", name="rm0")
                    nc.vector.tensor_reduce(rm0[:], pe0[:], axis=X, op=Alu.min)
                    nc.scalar.activation(
                        e0[:], pe0[:], Exp, bias=rm0[:], scale=-1.0,
                        accum_out=rs0[:],
                    )
                    ri0 = smpool.tile([128, 1], F32, tag="ri0", name="ri0")
                    nc.vector.reciprocal(ri0[:], rs0[:])
                    g0 = smpool.tile([128, 1], F32, tag="g0", name="g0")
                    nc.vector.scalar_tensor_tensor(
                        g0[:], ri0[:], 0.0, g128[:], op0=Alu.bypass, op1=Alu.mult
                    )
                    # diag block gets + I (identity fold)
                    nc.vector.scalar_tensor_tensor(
                        att16[0][:, 0:128], e0[:, 0:128], g0[:], ident[:],
                        op0=Alu.mult, op1=Alu.add,
                    )
                    nc.scalar.activation(
                        att16[0][:, 128:256], e0[:, 128:256], Copy, scale=g0[:]
                    )

                    # m=0 phase-2 operand first: phase 2 starts on it
                    # while the row-1 chain still runs on DVE/Act.
                    a16 = smpool.tile(
                        [128, 2, 128], F16, tag="aT0", name="aT0"
                    )
                    for jb in range(2):
                        pt = ptpool.tile([128, 128], F16, tag="pt", name="pt")
                        nc.tensor.transpose(
                            pt[:], att16[0][:, jb * 128:(jb + 1) * 128],
                            ident[:],
                        )
                        nc.vector.tensor_copy(a16[:, jb, :], pt[:])
                    aT.append(a16)

                    # phase 2 opens inside the pt scope: 512-wide psum tiles
                    # (pe 2 + pt 2 + po 4 = 8 banks).  Early matmuls also
                    # keep the PE HAM-warm through the row-1 chain.
                    po_ctx = tc.tile_pool(
                        name="po", bufs=4, space=bass.MemorySpace.PSUM
                    )
                    popool = po_ctx.__enter__()

                    def emit_p2(m, co):
                        outc = outpool.tile([128, WO], F16, tag="outc",
                                            name="outc")
                        for ci in range(WO // 512):
                            t0 = co * WO + ci * 512
                            po = popool.tile([128, 512], F32, tag="po",
                                             name="po")
                            for jb in range(2):
                                nc.tensor.matmul(
                                    po[:], aT[m][:, jb, :],
                                    xh[jb][:, t0:t0 + 512],
                                    start=(jb == 0), stop=(jb == 1),
                                )
                            dst = outc[:, ci * 512:(ci + 1) * 512]
                            if ci % 2 == 0:
                                nc.vector.tensor_copy(dst, po[:])
                            else:
                                nc.scalar.activation(dst, po[:], Copy)
                            # drain every 2048 cols (4 KiB rows)
                            if (ci + 1) % 4 == 0:
                                p0 = (ci - 3) * 512
                                nc.sync.dma_start(
                                    o_d.ap()[m, co][:, p0:p0 + 2048],
                                    outc[:, p0:p0 + 2048],
                                )

                    emit_p2(0, 0)

                    # row block 1: energy row = pe1b = [G01^T | G11]
                    rm1 = smpool.tile([128, 1], F32, tag="rm1", name="rm1")
                    nc.vector.tensor_reduce(rm1[:], pe1b[:], axis=X, op=Alu.min)
                    e1 = smpool.tile([128, C], F32, tag="e1", name="e1")
                    rs1 = smpool.tile([128, 1], F32, tag="rs1", name="rs1")
                    nc.scalar.activation(
                        e1[:], pe1b[:], Exp, bias=rm1[:], scale=-1.0,
                        accum_out=rs1[:],
                    )
                    ri1 = smpool.tile([128, 1], F32, tag="ri1", name="ri1")
                    nc.vector.reciprocal(ri1[:], rs1[:])
                    g1 = smpool.tile([128, 1], F32, tag="g1", name="g1")
                    nc.vector.scalar_tensor_tensor(
                        g1[:], ri1[:], 0.0, g128[:], op0=Alu.bypass, op1=Alu.mult
                    )
                    nc.scalar.activation(
                        att16[1][:, 0:128], e1[:, 0:128], Copy, scale=g1[:]
                    )
                    nc.vector.scalar_tensor_tensor(
                        att16[1][:, 128:256], e1[:, 128:256], g1[:], ident[:],
                        op0=Alu.mult, op1=Alu.add,
                    )
                    a16 = smpool.tile(
                        [128, 2, 128], F16, tag="aT1", name="aT1"
                    )
                    for jb in range(2):
                        pt = ptpool.tile([128, 128], F16, tag="pt", name="pt")
                        nc.tensor.transpose(
                            pt[:], att16[1][:, jb * 128:(jb + 1) * 128],
                            ident[:],
                        )
                        nc.vector.tensor_copy(a16[:, jb, :], pt[:])
                    aT.append(a16)

                    emit_p2(0, 1)
                    emit_p2(1, 0)
                    emit_p2(1, 1)
                    po_ctx.__exit__(None, None, None)

    nc.compile()
    return nc


_NC_CACHE = None


def _get_nc():
    global _NC_CACHE
    if _NC_CACHE is None:
        _NC_CACHE = _build_nc()
    return _NC_CACHE


def kernel(x, gamma):
    x = np.asarray(x)
    g = np.asarray(gamma, dtype=np.float32).reshape(-1)
    assert x.shape == (B, C, T), x.shape

    nc = _get_nc()
    xh = x.astype(np.float16).reshape(B, 2, 128, T)
    ident = np.eye(128, dtype=np.float16)
    gb = np.full((128, 1), g[0], dtype=np.float32)
    in_maps = []
    for b in range(B):
        im = {"identity": ident, "gamma_b": gb}
        off = 0
        for i, w in enumerate(SEGS):
            im[f"xseg{i}"] = np.ascontiguousarray(xh[b, :, :, off:off + w])
            off += w
        in_maps.append(im)

    trace = os.environ.get("KERNEL_TRACE", "0") == "1"
    res = run_bass_kernel_spmd(
        nc, in_maps, core_ids=list(range(N_CORES)), trace=trace
    )
    global LAST_RESULTS
    LAST_RESULTS = res
    # chunked output layout: [2, T//WO, 128, WO] -> [C, T]
    return np.stack(
        [
            r["out"].transpose(0, 2, 1, 3).reshape(C, T).astype(np.float32)
            for r in res.results
        ],
        axis=0,
    )



# revision 6
# speedup vs baseline: 1.0186x; 1.0186x over previous
"""Trainium2 Bass kernel for ChannelAttention1D.

Inputs (full): x (8, 256, 16384) f32, gamma (1,) f32.
  energy = einsum('bit,bjt->bij', x, x)
  att    = softmax(max_j(energy) - energy, axis=-1)
  out    = gamma * einsum('bij,bjt->bit', att, x) + x

Sharding: data-parallel over B across 8 NeuronCores (one batch per core).

HBM traffic is the roofline (memory regime): x is shipped once as fp16
(8 MiB/core) and the output is written as fp16 (8 MiB/core, upcast to f32
on the host).  The fp16 I/O rounding (~5e-4 max rel err) is far inside the
2e-2 gate; with gamma == 0 (the shipped input distribution) the folded
attention operand is exactly the identity, so out == fp16(x) bit-exact.

DMA layouts are chunked so descriptors stay large (descriptor generation
on the DGE caps DMA below the 358 GB/s wire rate when rows are only
4 KiB): input and output segments are separate DRAM tensors with 2-16 KiB
rows (small first input segment so compute starts early, small last
output segments to shorten the drain tail).  The host packs/unpacks.

Softmax epilogue (vs the earlier revision): G01^T is PE-transposed
directly into the same PSUM tile that accumulates G11, so row-block 1's
energy row is one contiguous [G01^T | G11] tile and its chain collapses
to a single rowmin + single exp (the row-0 shape) -- about 5 us of
serial cross-engine hops removed from the softmax window.

Energy matmuls are emitted one transpose h-group behind (not one
segment behind): the PE FIFO then always holds ready energy work instead
of data-starved next-segment transposes, which removes ~8 us of PE
head-of-line idle during the input stream and shrinks the post-stream
backlog to one h-group.  The last input segment is small (2048) so the
energy tail after the final byte is short.

The row-1 softmax chain is emitted between the two m=0 phase-2
blocks, so its DVE/Act ops fill those engines' slack while the PE runs
m=0 matmuls, and the aT[1] transposes slot between the m=0 and m=1
matmul streams with their input long ready -- no PE stall at either
block boundary.

Phase 2 is interleaved into the softmax epilogue: with 512-wide
phase-2 psum tiles (1 bank each), pe(2) + pt(2) + po(4) PSUM banks fit
simultaneously, so the first m=0 output block's matmuls/drains/writes
are emitted right after aT[0] -- phase 2 and the output write stream
start while the row-1 softmax chain still runs, and the early matmuls
keep the PE HAM-warm (the dummy clock-hold transposes are gone).

Per-core pipeline (C=256, T=16384):
  phase 1: sync-ring DMA streams x fp16 segments.  PE transposes 128x128
           blocks into PSUM (fp16); DVE (m=0) and Act (m=1) copy them to
           SBUF downcasting to fp8e4m3 in DoubleRow-pair layout
           xtp [128 tp, q, 2 kt, 2 m, 128 c].  Energy accumulates with
           fp8 DoubleRow matmuls (K=256 per pass): only G00|G01 (pe0) and
           G11 (pe1) are computed; G10 = G01^T by symmetry.
  softmax: att = exp(rowmin - energy) / rowsum (== softmax(rowmax -
           energy)); G01^T is reconstructed with an fp16 PE transpose.
           A = gamma*att/rowsum + I is formed directly (identity folded
           into the operand), so phase 2 needs no residual add.
  phase 2: out = A.T-transposed matmuls @ x straight from the resident
           natural x tiles (fp16), PSUM drained to fp16 by DVE/Act
           alternately, 16 KiB-row writeback.
"""

import os

# Neuron cores can be left in a degraded (down-clocked ~20%) state by prior
# runs; a core reset at runtime init restores full clocks (see trn2 pitfalls).
os.environ.setdefault("NEURON_RT_RESET_CORES", "1")

import numpy as np

import concourse.bacc as bacc
import concourse.bass as bass
import concourse.mybir as mybir
import concourse.tile as tile
from concourse.bass_utils import run_bass_kernel_spmd

F32 = mybir.dt.float32
F16 = mybir.dt.float16
F8 = mybir.dt.float8e4

B = 8
C = 256
T = 16384
N_CORES = 8
SEGS = [2048, 4096, 4096, 4096, 2048]   # in segments (fp16 cols) per m
QMAX = max(SEGS) // 256                 # xtp tile q capacity (padded)
W2 = 1024            # phase-2 psum tile width (2 fp32 PSUM banks)
WO = 8192            # phase-2 output staging width (16 KiB rows)

LAST_RESULTS = None  # BassKernelResults of the most recent run (for test.py)


def _build_nc():
    nc = bacc.Bacc(
        "TRN2",
        target_bir_lowering=False,
        debug=False,
        enable_asserts=False,
        num_devices=N_CORES,
    )
    seg_d = [
        nc.dram_tensor(f"xseg{i}", [2, 128, w], F16, kind="ExternalInput")
        for i, w in enumerate(SEGS)
    ]
    id_d = nc.dram_tensor("identity", [128, 128], F16, kind="ExternalInput")
    g_d = nc.dram_tensor("gamma_b", [128, 1], F32, kind="ExternalInput")
    o_d = nc.dram_tensor("out", [2, T // WO, 128, WO], F16, kind="ExternalOutput")

    Exp = mybir.ActivationFunctionType.Exp
    Copy = mybir.ActivationFunctionType.Copy
    Alu = mybir.AluOpType
    X = mybir.AxisListType.X
    DR = mybir.MatmulPerfMode.DoubleRow
    NQ = T // 256

    with tile.TileContext(nc) as tc:
        with (
            tc.tile_pool(name="xh", bufs=1) as xhpool,
            tc.tile_pool(name="xtp", bufs=3) as xtppool,
            tc.tile_pool(name="sm", bufs=1) as smpool,
            tc.tile_pool(name="outp", bufs=4) as outpool,
        ):
            ident = smpool.tile([128, 128], F16, tag="ident", name="ident")
            nc.scalar.dma_start(ident[:], id_d.ap())
            g128 = smpool.tile([128, 1], F32, tag="g128", name="g128")
            nc.scalar.dma_start(g128[:], g_d.ap())

            # Resident fp16 x (natural layout), one tile per 128-row block.
            xh = [
                xhpool.tile([128, T], F16, tag=f"xh{m}", name=f"xh{m}")
                for m in range(2)
            ]

            with tc.tile_pool(
                name="pe", bufs=1, space=bass.MemorySpace.PSUM
            ) as pepool:
                ptx_ctx = tc.tile_pool(
                    name="ptx", bufs=4, space=bass.MemorySpace.PSUM
                )
                ptxpool = ptx_ctx.__enter__()
                pe0 = pepool.tile([128, C], F32, tag="pe0", name="pe0")
                pe1b = pepool.tile([128, C], F32, tag="pe1b", name="pe1b")
                pe1 = pe1b[:, 128:256]


                # ---- phase 1: stream in, PE-transpose, fp8 DR energy ----
                # energy matmuls run one segment behind the transposes so the
                # PE never stalls waiting for the current segment's DVE/Act
                # psum->sbuf copies
                k = 0
                off = 0
                pending = []  # [(xtp, q0, nq), ...] per ready h-group

                def emit_energy(xtp, q0, nq):
                    nonlocal k
                    for q in range(q0, q0 + nq):
                        st = k == 0
                        sp = k == NQ - 1
                        w0 = xtp[:, q, :, 0, :]
                        w1 = xtp[:, q, :, 1, :]
                        rhs_all = xtp[:, q].rearrange("p kt m c -> p kt (m c)")
                        nc.tensor.matmul(
                            pe0[:], w0, rhs_all, start=st, stop=sp, perf_mode=DR
                        )
                        nc.tensor.matmul(
                            pe1, w1, w1, start=st, stop=sp, perf_mode=DR
                        )
                        k += 1

                for si, w in enumerate(SEGS):
                    for m in range(2):
                        nc.sync.dma_start(
                            xh[m][:, off:off + w], seg_d[si].ap()[m]
                        )
                    # xtp[p, q, kt, m, c] = x[m*128+c, off + (2q+kt)*128 + p]
                    xtp = xtppool.tile(
                        [128, QMAX, 2, 2, 128], F8, tag="xtp", name=f"xtp{si}"
                    )
                    ntb = w // 128
                    for h in range((ntb + 7) // 8):
                        tbs = min(8, ntb - h * 8)
                        for m in range(2):
                            ptx = ptxpool.tile(
                                [128, 8, 128], F16, tag="ptx",
                                name=f"ptx{m}_{si}_{h}"
                            )
                            for tbl in range(tbs):
                                tb = h * 8 + tbl
                                nc.tensor.transpose(
                                    ptx[:, tbl, :],
                                    xh[m][:, off + tb * 128:off + (tb + 1) * 128],
                                    ident[:],
                                )
                            src = ptx[:, 0:tbs, :].rearrange(
                                "p (q kt) c -> p q kt c", kt=2
                            )
                            dst = xtp[:, h * 4:h * 4 + tbs // 2, :, m, :]
                            if m == 0:
                                nc.vector.tensor_copy(dst, src)
                            else:
                                nc.scalar.activation(dst, src, Copy)
                        pending.append((xtp, h * 4, tbs // 2))
                        if len(pending) > 1:
                            emit_energy(*pending.pop(0))
                    off += w
                for p in pending:
                    emit_energy(*p)

                # G01^T goes straight into pe1b[:, 0:128] so row-1''s
                # energy row is one contiguous psum tile (one reduce+exp)
                s01 = smpool.tile([128, 128], F32, tag="s01", name="s01")
                nc.vector.tensor_copy(s01[:], pe0[:, 128:256])
                id32 = smpool.tile([128, 128], F32, tag="id32", name="id32")
                nc.scalar.activation(id32[:], ident[:], Copy)
                nc.tensor.transpose(pe1b[:, 0:128], s01[:], id32[:])

                ptx_ctx.__exit__(None, None, None)

                # ---- softmax epilogue; A = gamma*att/rowsum + I ----
                att16 = [
                    smpool.tile([128, C], F16, tag=f"a{m}", name=f"a{m}")
                    for m in range(2)
                ]
                aT = []  # fp16 A.T operands for phase 2, [128 j, 2 jb, 128 i]
                with tc.tile_pool(
                    name="pt", bufs=2, space=bass.MemorySpace.PSUM
                ) as ptpool:
                    # row block 0: energy row = pe0 = [G00 | G01]
                    e0 = smpool.tile([128, C], F32, tag="e0", name="e0")
                    rs0 = smpool.tile([128, 1], F32, tag="rs0", name="rs0")
                    rm0 = smpool.tile([128, 1], F32, tag="rm0", name="rm0")
                    nc.vector.tensor_reduce(rm0[:], pe0[:], axis=X, op=Alu.min)
                    nc.scalar.activation(
                        e0[:], pe0[:], Exp, bias=rm0[:], scale=-1.0,
                        accum_out=rs0[:],
                    )
                    ri0 = smpool.tile([128, 1], F32, tag="ri0", name="ri0")
                    nc.vector.reciprocal(ri0[:], rs0[:])
                    g0 = smpool.tile([128, 1], F32, tag="g0", name="g0")
                    nc.vector.scalar_tensor_tensor(
                        g0[:], ri0[:], 0.0, g128[:], op0=Alu.bypass, op1=Alu.mult
                    )
                    # diag block gets + I (identity fold)
                    nc.vector.scalar_tensor_tensor(
                        att16[0][:, 0:128], e0[:, 0:128], g0[:], ident[:],
                        op0=Alu.mult, op1=Alu.add,
                    )
                    nc.scalar.activation(
                        att16[0][:, 128:256], e0[:, 128:256], Copy, scale=g0[:]
                    )

                    # m=0 phase-2 operand first: phase 2 starts on it
                    # while the row-1 chain still runs on DVE/Act.
                    a16 = smpool.tile(
                        [128, 2, 128], F16, tag="aT0", name="aT0"
                    )
                    for jb in range(2):
                        pt = ptpool.tile([128, 128], F16, tag="pt", name="pt")
                        nc.tensor.transpose(
                            pt[:], att16[0][:, jb * 128:(jb + 1) * 128],
                            ident[:],
                        )
                        nc.vector.tensor_copy(a16[:, jb, :], pt[:])
                    aT.append(a16)

                    # phase 2 opens inside the pt scope: 512-wide psum tiles
                    # (pe 2 + pt 2 + po 4 = 8 banks).  Early matmuls also
                    # keep the PE HAM-warm through the row-1 chain.
                    po_ctx = tc.tile_pool(
                        name="po", bufs=4, space=bass.MemorySpace.PSUM
                    )
                    popool = po_ctx.__enter__()

                    def emit_p2(m, co):
                        outc = outpool.tile([128, WO], F16, tag="outc",
                                            name="outc")
                        for ci in range(WO // 512):
                            t0 = co * WO + ci * 512
                            po = popool.tile([128, 512], F32, tag="po",
                                             name="po")
                            for jb in range(2):
                                nc.tensor.matmul(
                                    po[:], aT[m][:, jb, :],
                                    xh[jb][:, t0:t0 + 512],
                                    start=(jb == 0), stop=(jb == 1),
                                )
                            dst = outc[:, ci * 512:(ci + 1) * 512]
                            if ci % 2 == 0:
                                nc.vector.tensor_copy(dst, po[:])
                            else:
                                nc.scalar.activation(dst, po[:], Copy)
                            # drain every 2048 cols (4 KiB rows)
                            if (ci + 1) % 4 == 0:
                                p0 = (ci - 3) * 512
                                nc.sync.dma_start(
                                    o_d.ap()[m, co][:, p0:p0 + 2048],
                                    outc[:, p0:p0 + 2048],
                                )

                    emit_p2(0, 0)

                    # row block 1: energy row = pe1b = [G01^T | G11]
                    rm1 = smpool.tile([128, 1], F32, tag="rm1", name="rm1")
                    nc.vector.tensor_reduce(rm1[:], pe1b[:], axis=X, op=Alu.min)
                    e1 = smpool.tile([128, C], F32, tag="e1", name="e1")
                    rs1 = smpool.tile([128, 1], F32, tag="rs1", name="rs1")
                    nc.scalar.activation(
                        e1[:], pe1b[:], Exp, bias=rm1[:], scale=-1.0,
                        accum_out=rs1[:],
                    )
                    ri1 = smpool.tile([128, 1], F32, tag="ri1", name="ri1")
                    nc.vector.reciprocal(ri1[:], rs1[:])
                    g1 = smpool.tile([128, 1], F32, tag="g1", name="g1")
                    nc.vector.scalar_tensor_tensor(
                        g1[:], ri1[:], 0.0, g128[:], op0=Alu.bypass, op1=Alu.mult
                    )
                    nc.scalar.activation(
                        att16[1][:, 0:128], e1[:, 0:128], Copy, scale=g1[:]
                    )
                    nc.vector.scalar_tensor_tensor(
                        att16[1][:, 128:256], e1[:, 128:256], g1[:], ident[:],
                        op0=Alu.mult, op1=Alu.add,
                    )
                    emit_p2(0, 1)

                    a16 = smpool.tile(
                        [128, 2, 128], F16, tag="aT1", name="aT1"
                    )
                    for jb in range(2):
                        pt = ptpool.tile([128, 128], F16, tag="pt", name="pt")
                        nc.tensor.transpose(
                            pt[:], att16[1][:, jb * 128:(jb + 1) * 128],
                            ident[:],
                        )
                        nc.vector.tensor_copy(a16[:, jb, :], pt[:])
                    aT.append(a16)

                    emit_p2(1, 0)
                    emit_p2(1, 1)
                    po_ctx.__exit__(None, None, None)

    nc.compile()
    return nc


_NC_CACHE = None


def _get_nc():
    global _NC_CACHE
    if _NC_CACHE is None:
        _NC_CACHE = _build_nc()
    return _NC_CACHE


def kernel(x, gamma):
    x = np.asarray(x)
    g = np.asarray(gamma, dtype=np.float32).reshape(-1)
    assert x.shape == (B, C, T), x.shape

    nc = _get_nc()
    xh = x.astype(np.float16).reshape(B, 2, 128, T)
    ident = np.eye(128, dtype=np.float16)
    gb = np.full((128, 1), g[0], dtype=np.float32)
    in_maps = []
    for b in range(B):
        im = {"identity": ident, "gamma_b": gb}
        off = 0
        for i, w in enumerate(SEGS):
            im[f"xseg{i}"] = np.ascontiguousarray(xh[b, :, :, off:off + w])
            off += w
        in_maps.append(im)

    trace = os.environ.get("KERNEL_TRACE", "0") == "1"
    res = run_bass_kernel_spmd(
        nc, in_maps, core_ids=list(range(N_CORES)), trace=trace
    )
    global LAST_RESULTS
    LAST_RESULTS = res
    # chunked output layout: [2, T//WO, 128, WO] -> [C, T]
    return np.stack(
        [
            r["out"].transpose(0, 2, 1, 3).reshape(C, T).astype(np.float32)
            for r in res.results
        ],
        axis=0,
    )

